# revision 2
# baseline (speedup 1.0000x reference)
"""GATv2 (2-layer) Trainium2 Bass kernel, 8-core SPMD.

Strategy:
- Edges sorted by destination, grouped into 128-node dst blocks; contiguous
  block ranges assigned to the 8 cores with balanced edge counts.
- Per 128-edge tile: src features fetched with an indirect DMA row-gather
  from a full node table; dst features expanded from a streamed per-block
  node tile via a one-hot PE matmul (no dst gather).
- Segment softmax without max subtraction (exact: logits are O(1)); the
  denominator is accumulated with a ones-column matmul and applied after
  aggregation.
- leaky_relu(x, 0.2) == 0.6x + 0.4|x| lets logits be computed as two
  weighted row-reductions (no per-edge [H,C] activations materialized).
- Two launches (layer 1 -> host concat of per-core h slices -> layer 2).
"""

import json
import numpy as np

import concourse.bass as bass
import concourse.mybir as mybir
from concourse.tile import TileContext, ScopedClock
from concourse.bass_utils import run_bass_kernel_spmd
from concourse.masks import make_identity

# ----------------------------------------------------------------------------
# Workarounds for the walrus build in this container: at most ONE sync-wait
# per instruction. Extra waits are peeled onto NoOps inserted just before.
# ----------------------------------------------------------------------------
_MAXW = 1
_split_counter = [0]


def _patched_drain_and_barrier(self, tick_clock, wait_clock):
    d0 = self.nc.sync.drain()
    wait_clock.add_sem_waits(d0.ins, ScopedClock({None: tick_clock.global_clock}))
    waits = list(d0.ins.sync_info.on_wait)
    if len(waits) > _MAXW:
        del d0.ins.sync_info.on_wait[_MAXW:]
        rest = waits[_MAXW:]
        for i in range(0, len(rest), _MAXW):
            d = self.nc.sync.drain()
            if d.ins.sync_info is None:
                d.ins.sync_info = mybir.SyncInfo(on_update=[], on_wait=[])
            d.ins.sync_info.on_wait.extend(rest[i:i + _MAXW])
    self.nc.all_engine_barrier()
    popped = self.nc._tile_sem_poison_stack.pop()
    assert popped is self._sem_poison
    self.nc.clear_and_free_semaphores(list(self.sems.allocated().values()))
    self.nc.all_engine_barrier()


def _fix_bir_json(data: bytes) -> bytes:
    m = json.loads(data)
    changed = False
    for f in m.get("functions", []):
        for b in f.get("blocks", []):
            insts = b.get("instructions")
            if not insts:
                continue
            out = []
            for inst in insts:
                si = inst.get("sync_info") or {}
                waits = si.get("on_wait") or []
                if len(waits) > 1:
                    for w in waits[:-1]:
                        _split_counter[0] += 1
                        out.append({
                            "name": f"I-sw{_split_counter[0]}",
                            "opcode": "NoOp",
                            "engine": inst.get("engine"),
                            "ins": [], "outs": [],
                            "sync_info": {"on_update": [], "on_wait": [w]},
                        })
                    si["on_wait"] = [waits[-1]]
                    changed = True
                out.append(inst)
            b["instructions"] = out
    if not changed:
        return data
    return json.dumps(m).encode()


def _install_fixes():
    TileContext._drain_and_barrier = _patched_drain_and_barrier
    if not getattr(bass.Bass, "_tilefix_json", False):
        orig = bass.Bass.to_json_bytes

        def to_json_bytes(self, *a, **k):
            return _fix_bir_json(orig(self, *a, **k))

        bass.Bass.to_json_bytes = to_json_bytes
        bass.Bass._tilefix_json = True


_install_fixes()

# ----------------------------------------------------------------------------
N_NODES = 100_000
N_EDGES = 1_600_000
F_IN = 128
H1, C1 = 2, 64
H2, C2 = 1, 64
NCORES = 8
P = 128
NBLK_GLOBAL = (N_NODES + P - 1) // P        # 782
NPAD = NBLK_GLOBAL * P                      # 100096
CH = 8                                      # stream chunk (tiles)
F32 = mybir.dt.float32
I32 = mybir.dt.int32


def _rep(v):
    """Replicate a 1-D row across 128 partitions -> [128, len] f32."""
    v = np.asarray(v, np.float32).reshape(1, -1)
    return np.ascontiguousarray(np.repeat(v, P, axis=0))


def _prep_edges(edge_index):
    src = np.asarray(edge_index[0], np.int64)
    dst = np.asarray(edge_index[1], np.int64)
    E = src.shape[0]
    order = np.argsort(dst, kind="stable")
    src_s = src[order].astype(np.int32)
    dst_s = dst[order].astype(np.int32)
    gb = dst_s // P                                    # global block id, sorted
    blk_cnt = np.bincount(gb, minlength=NBLK_GLOBAL)
    # contiguous block ranges per core, balanced by edge count
    cum = np.cumsum(blk_cnt)
    bounds = [0]
    for k in range(1, NCORES):
        t = E * k / NCORES
        b = int(np.searchsorted(cum, t))
        bounds.append(max(min(b, NBLK_GLOBAL - (NCORES - k)), bounds[-1] + 1))
    bounds.append(NBLK_GLOBAL)
    core_rng = [(bounds[k], bounds[k + 1]) for k in range(NCORES)]
    NBLKC = max(b1 - b0 for b0, b1 in core_rng)
    # tiles per block slot = max over cores
    T_list = []
    for s in range(NBLKC):
        mx = 1
        for b0, b1 in core_rng:
            if b0 + s < b1:
                mx = max(mx, (int(blk_cnt[b0 + s]) + P - 1) // P)
        T_list.append(mx + 1)  # leading all-pad tile per block (see _build_layer)
    T_total = sum(T_list)
    Tpad = ((T_total + CH - 1) // CH) * CH
    blk_start = np.concatenate([[0], cum]).astype(np.int64)  # edge offset per block
    idx_st = np.zeros((NCORES, P, Tpad), np.int32)
    ea_dummy = np.zeros((NCORES, P, Tpad), np.float32)
    dr_st = np.full((NCORES, P, Tpad), -1.0, np.float32)
    eorder = np.empty((NCORES,), object)
    slots = np.empty((NCORES,), object)
    for k in range(NCORES):
        b0, b1 = core_rng[k]
        col = 0
        ords, slts = [], []
        for s in range(NBLKC):
            b = b0 + s
            if b < b1:
                e0, e1 = int(blk_start[b]), int(blk_start[b + 1])
                n = e1 - e0
                sl = np.arange(n, dtype=np.int64)
                p_ = sl % P
                c_ = col + 1 + sl // P  # skip the leading pad tile
                idx_st[k, p_, c_] = src_s[e0:e1]
                dr_st[k, p_, c_] = (dst_s[e0:e1] % P).astype(np.float32)
                ords.append(order[e0:e1])
                slts.append((p_, c_))
            col += T_list[s]
        eorder[k] = ords
        slots[k] = slts
    return dict(core_rng=core_rng, NBLKC=NBLKC, T_list=T_list, Tpad=Tpad,
                idx_st=idx_st, dr_st=dr_st, ea_shape=ea_dummy.shape,
                eorder=eorder, slots=slots)


def _fill_ea(prep, edge_attr):
    ea = np.asarray(edge_attr, np.float32).reshape(-1)
    out = np.zeros((NCORES, P, prep["Tpad"]), np.float32)
    for k in range(NCORES):
        for (p_, c_), orig in zip(prep["slots"][k], prep["eorder"][k]):
            out[k, p_, c_] = ea[orig]
    return out


def _build_layer(COUT, H, NBLKC, T_list, Tpad, do_relu):
    """One GATv2 layer. Inputs (per core): xT [128,NPAD], xTl [128,NBLKC*128],
    streams idx/ea/dr, weights/consts. Output h_out [NBLKC*128, COUT]."""
    C = COUT // H
    nc = bass.Bass()
    xT = nc.dram_tensor("xT", [P, NPAD], F32, kind="ExternalInput")
    xTl = nc.dram_tensor("xTl", [P, NBLKC * P], F32, kind="ExternalInput")
    idx_d = nc.dram_tensor("idx", [P, Tpad], I32, kind="ExternalInput")
    ea_d = nc.dram_tensor("ea", [P, Tpad], F32, kind="ExternalInput")
    dr_d = nc.dram_tensor("dr", [P, Tpad], F32, kind="ExternalInput")
    Wl_d = nc.dram_tensor("Wl", [P, COUT], F32, kind="ExternalInput")
    Wr_d = nc.dram_tensor("Wr", [P, COUT], F32, kind="ExternalInput")
    vV_d = nc.dram_tensor("vV", [P, COUT], F32, kind="ExternalInput")
    attV_d = nc.dram_tensor("attV", [P, COUT], F32, kind="ExternalInput")
    biasV_d = nc.dram_tensor("biasV", [P, COUT], F32, kind="ExternalInput")
    bWlV_d = nc.dram_tensor("bWlV", [P, COUT], F32, kind="ExternalInput")
    bWrV_d = nc.dram_tensor("bWrV", [P, COUT], F32, kind="ExternalInput")
    iotaV_d = nc.dram_tensor("iotaV", [P, P], F32, kind="ExternalInput")
    onesV_d = nc.dram_tensor("onesV", [P, 1], F32, kind="ExternalInput")
    h_out = nc.dram_tensor("h_out", [NBLKC * P, COUT], F32, kind="ExternalOutput")
    xl_full = nc.dram_tensor("xl_full", [NPAD, COUT], F32)
    xr_loc = nc.dram_tensor("xr_loc", [NBLKC * P, COUT], F32)
    AL = mybir.AluOpType
    AF = mybir.ActivationFunctionType

    with TileContext(nc) as tc:
        with (
            tc.tile_pool(name="const", bufs=1) as cp,
            tc.tile_pool(name="sbuf", bufs=4) as pool,
            tc.tile_pool(name="st", bufs=3) as sp,
            tc.tile_pool(name="eps", bufs=2) as ep,
            tc.tile_pool(name="pd", bufs=2, space="PSUM") as ppd,
            tc.tile_pool(name="pt", bufs=2, space="PSUM") as ppt,
            tc.tile_pool(name="px", bufs=2, space="PSUM") as ppx,
            tc.tile_pool(name="po", bufs=2, space="PSUM") as ppo,
        ):
            Wl = cp.tile([P, COUT], F32); nc.sync.dma_start(out=Wl[:], in_=Wl_d[:])
            Wr = cp.tile([P, COUT], F32); nc.sync.dma_start(out=Wr[:], in_=Wr_d[:])
            vV = cp.tile([P, COUT], F32); nc.sync.dma_start(out=vV[:], in_=vV_d[:])
            attV = cp.tile([P, COUT], F32); nc.sync.dma_start(out=attV[:], in_=attV_d[:])
            biasV = cp.tile([P, COUT], F32); nc.sync.dma_start(out=biasV[:], in_=biasV_d[:])
            bWlV = cp.tile([P, COUT], F32); nc.sync.dma_start(out=bWlV[:], in_=bWlV_d[:])
            bWrV = cp.tile([P, COUT], F32); nc.sync.dma_start(out=bWrV[:], in_=bWrV_d[:])
            iotaV = cp.tile([P, P], F32); nc.sync.dma_start(out=iotaV[:], in_=iotaV_d[:])
            onesV = cp.tile([P, 1], F32); nc.sync.dma_start(out=onesV[:], in_=onesV_d[:])
            ident = cp.tile([P, P], F32); make_identity(nc, ident[:])

            # dense: xl_full = (xT.T @ Wl) + b_l ; xr_loc likewise from xTl
            for j in range(NPAD // P):
                xt = pool.tile([P, P], F32, tag="xt")
                nc.sync.dma_start(out=xt[:], in_=xT[:, j * P:(j + 1) * P])
                pd = ppd.tile([P, COUT], F32, space="PSUM")
                nc.tensor.matmul(pd[:], lhsT=xt[:], rhs=Wl[:], start=True, stop=True)
                xls = pool.tile([P, COUT], F32, tag="xls")
                nc.vector.tensor_tensor(out=xls[:], in0=pd[:], in1=bWlV[:], op=AL.add)
                nc.sync.dma_start(out=xl_full[j * P:(j + 1) * P, :], in_=xls[:])
            for s in range(NBLKC):
                xt = pool.tile([P, P], F32, tag="xt")
                nc.sync.dma_start(out=xt[:], in_=xTl[:, s * P:(s + 1) * P])
                pd = ppd.tile([P, COUT], F32, space="PSUM")
                nc.tensor.matmul(pd[:], lhsT=xt[:], rhs=Wr[:], start=True, stop=True)
                xrs = pool.tile([P, COUT], F32, tag="xls")
                nc.vector.tensor_tensor(out=xrs[:], in0=pd[:], in1=bWrV[:], op=AL.add)
                nc.sync.dma_start(out=xr_loc[s * P:(s + 1) * P, :], in_=xrs[:])

            # edge phase
            g = 0
            for s in range(NBLKC):
                xrb = pool.tile([P, COUT], F32, tag="xrb")
                nc.sync.dma_start(out=xrb[:], in_=xr_loc[s * P:(s + 1) * P, :])
                psO = ppo.tile([P, COUT + H], F32, space="PSUM")
                for t in range(T_list[s]):
                    if g % CH == 0:
                        idxc = sp.tile([P, CH], I32, tag="idxc")
                        nc.sync.dma_start(out=idxc[:], in_=idx_d[:, g:g + CH])
                        eac = sp.tile([P, CH], F32, tag="eac")
                        nc.sync.dma_start(out=eac[:], in_=ea_d[:, g:g + CH])
                        drc = sp.tile([P, CH], F32, tag="drc")
                        nc.sync.dma_start(out=drc[:], in_=dr_d[:, g:g + CH])
                    c = g % CH
                    msgA = pool.tile([P, COUT], F32, tag="msgA")
                    nc.gpsimd.indirect_dma_start(
                        out=msgA[:], out_offset=None, in_=xl_full[:, :],
                        in_offset=bass.IndirectOffsetOnAxis(ap=idxc[:, c:c + 1], axis=0))
                    S01 = pool.tile([P, P], F32, tag="S01")
                    nc.vector.tensor_scalar(out=S01[:], in0=iotaV[:],
                                            scalar1=drc[:, c:c + 1], scalar2=None,
                                            op0=AL.is_equal)
                    pT = ppt.tile([P, P], F32, space="PSUM")
                    nc.tensor.transpose(out=pT[:], in_=S01[:], identity=ident[:])
                    selD = pool.tile([P, P], F32, tag="selD")
                    nc.scalar.copy(selD[:], pT[:])
                    pXR = ppx.tile([P, COUT], F32, space="PSUM")
                    nc.tensor.matmul(pXR[:], lhsT=selD[:], rhs=xrb[:], start=True, stop=True)
                    m = pool.tile([P, COUT], F32, tag="m")
                    nc.vector.scalar_tensor_tensor(out=m[:], in0=vV[:],
                                                   scalar=eac[:, c:c + 1], in1=pXR[:],
                                                   op0=AL.mult, op1=AL.add)
                    nc.vector.tensor_tensor(out=m[:], in0=m[:], in1=msgA[:], op=AL.add)
                    tabs = pool.tile([P, COUT], F32, tag="tabs")
                    nc.scalar.activation(tabs[:], m[:], AF.Abs)
                    q = pool.tile([P, COUT], F32, tag="q")
                    nc.vector.tensor_tensor(out=q[:], in0=m[:], in1=attV[:], op=AL.mult)
                    lin = pool.tile([P, H], F32, tag="lin")
                    nc.vector.tensor_reduce(out=lin[:], in_=q[:].rearrange("p (h c) -> p h c", h=H),
                                            axis=mybir.AxisListType.X, op=AL.add)
                    u = pool.tile([P, COUT], F32, tag="u")
                    nc.vector.tensor_tensor(out=u[:], in0=tabs[:], in1=attV[:], op=AL.mult)
                    ur = pool.tile([P, H], F32, tag="ur")
                    nc.vector.tensor_reduce(out=ur[:], in_=u[:].rearrange("p (h c) -> p h c", h=H),
                                            axis=mybir.AxisListType.X, op=AL.add)
                    logit = pool.tile([P, H], F32, tag="logit")
                    nc.vector.tensor_scalar(out=logit[:], in0=lin[:], scalar1=0.6,
                                            scalar2=None, op0=AL.mult)
                    nc.vector.scalar_tensor_tensor(out=logit[:], in0=ur[:], scalar=0.4,
                                                   in1=logit[:], op0=AL.mult, op1=AL.add)
                    ex = pool.tile([P, H], F32, tag="ex")
                    nc.scalar.activation(ex[:], logit[:], AF.Exp)
                    first, last = (t == 0), (t == T_list[s] - 1)
                    for h in range(H):
                        Sh = pool.tile([P, P], F32, tag=f"Sh{h}")
                        nc.scalar.activation(Sh[:], S01[:], AF.Copy,
                                             bias=0.0, scale=ex[:, h:h + 1])
                        nc.tensor.matmul(psO[:, h * C:(h + 1) * C], lhsT=Sh[:],
                                         rhs=msgA[:, h * C:(h + 1) * C],
                                         start=first, stop=last)
                        nc.tensor.matmul(psO[:, COUT + h:COUT + h + 1], lhsT=Sh[:],
                                         rhs=onesV[:], start=first, stop=last)
                    g += 1
                den = ep.tile([P, H], F32, tag="den")
                nc.vector.tensor_scalar_max(den[:], psO[:, COUT:COUT + H], 1e-30)
                dinv = ep.tile([P, H], F32, tag="dinv")
                nc.vector.reciprocal(dinv[:], den[:])
                hsb = ep.tile([P, COUT], F32, tag="hsb")
                for h in range(H):
                    nc.vector.tensor_scalar(out=hsb[:, h * C:(h + 1) * C],
                                            in0=psO[:, h * C:(h + 1) * C],
                                            scalar1=dinv[:, h:h + 1], scalar2=None,
                                            op0=AL.mult)
                nc.vector.tensor_tensor(out=hsb[:], in0=hsb[:], in1=biasV[:], op=AL.add)
                if do_relu:
                    nc.vector.tensor_scalar_max(hsb[:], hsb[:], 0.0)
                nc.sync.dma_start(out=h_out[s * P:(s + 1) * P, :], in_=hsb[:])
    return nc


def _run_layer(nc, per_core_ins):
    res = run_bass_kernel_spmd(nc, per_core_ins, core_ids=list(range(NCORES)))
    return [r["h_out"] for r in res.results]


def _layer_inputs(prep, ea_st, xT_full, Wl, bl, Wr, br, We, att, bias, COUT, H):
    iotaV = _rep(np.arange(P, dtype=np.float32))
    onesV = np.ones((P, 1), np.float32)
    common = dict(
        Wl=np.ascontiguousarray(Wl.astype(np.float32)),
        Wr=np.ascontiguousarray(Wr.astype(np.float32)),
        vV=_rep(We.reshape(-1)),
        attV=_rep(att.reshape(-1)),
        biasV=_rep(bias),
        bWlV=_rep(bl),
        bWrV=_rep(br),
        iotaV=iotaV,
        onesV=onesV,
        xT=xT_full,
    )
    per_core = []
    NBLKC = prep["NBLKC"]
    for k in range(NCORES):
        b0, b1 = prep["core_rng"][k]
        xTl = np.zeros((P, NBLKC * P), np.float32)
        lo, hi = b0 * P, min(b1 * P, NPAD)
        w = hi - lo
        xTl[:, :w] = xT_full[:, lo:hi]
        d = dict(common)
        d["xTl"] = xTl
        d["idx"] = prep["idx_st"][k]
        d["ea"] = ea_st[k]
        d["dr"] = prep["dr_st"][k]
        per_core.append(d)
    return per_core


def _assemble(prep, outs, COUT):
    full = np.zeros((NPAD, COUT), np.float32)
    for k in range(NCORES):
        b0, b1 = prep["core_rng"][k]
        n = (b1 - b0) * P
        full[b0 * P: b1 * P, :] = outs[k][:n, :]
    return full


def kernel(x, edge_index, edge_attr,
           W1_l, b1_l, W1_r, b1_r, W1_e, att1, bias1,
           W2_l, b2_l, W2_r, b2_r, W2_e, att2, bias2):
    x = np.asarray(x, np.float32)
    prep = _prep_edges(np.asarray(edge_index))
    ea_st = _fill_ea(prep, edge_attr)

    xpad = np.zeros((NPAD, F_IN), np.float32)
    xpad[:N_NODES] = x
    xT = np.ascontiguousarray(xpad.T)

    NBLKC, T_list, Tpad = prep["NBLKC"], prep["T_list"], prep["Tpad"]

    nc1 = _build_layer(H1 * C1, H1, NBLKC, T_list, Tpad, do_relu=True)
    ins1 = _layer_inputs(prep, ea_st, xT, W1_l, b1_l, W1_r, b1_r, W1_e, att1,
                         bias1, H1 * C1, H1)
    h_slices = _run_layer(nc1, ins1)
    h_full = _assemble(prep, h_slices, H1 * C1)
    hT = np.ascontiguousarray(h_full.T)

    nc2 = _build_layer(H2 * C2, H2, NBLKC, T_list, Tpad, do_relu=False)
    ins2 = _layer_inputs(prep, ea_st, hT, W2_l, b2_l, W2_r, b2_r, W2_e, att2,
                         bias2, H2 * C2, H2)
    o_slices = _run_layer(nc2, ins2)
    out_full = _assemble(prep, o_slices, H2 * C2)
    return out_full[:N_NODES].astype(np.float32)


# revision 3
# speedup vs baseline: 3.0335x; 3.0335x over previous
"""GATv2 (2-layer) Trainium2 Bass kernel, 8-core SPMD.

Strategy:
- Edges sorted by destination, grouped into 128-node dst blocks; contiguous
  block ranges assigned to the 8 cores with balanced edge counts.
- Per 128-edge tile: src features fetched with an indirect DMA row-gather
  from a full node table; dst features expanded from a streamed per-block
  node tile via a one-hot PE matmul (no dst gather).
- Segment softmax without max subtraction (exact: logits are O(1)); the
  denominator is accumulated with a ones-column matmul and applied after
  aggregation.
- leaky_relu(x, 0.2) == 0.6x + 0.4|x| lets logits be computed as two
  weighted row-reductions (no per-edge [H,C] activations materialized).
- Two launches (layer 1 -> host concat of per-core h slices -> layer 2).
"""

import json
import numpy as np

import concourse.bass as bass
import concourse.mybir as mybir
from concourse.tile import TileContext, ScopedClock
from concourse.bass_utils import run_bass_kernel_spmd
from concourse.masks import make_identity

# ----------------------------------------------------------------------------
# Workarounds for the walrus build in this container: at most ONE sync-wait
# per instruction. Extra waits are peeled onto NoOps inserted just before.
# ----------------------------------------------------------------------------
_MAXW = 1
_split_counter = [0]


def _patched_drain_and_barrier(self, tick_clock, wait_clock):
    d0 = self.nc.sync.drain()
    wait_clock.add_sem_waits(d0.ins, ScopedClock({None: tick_clock.global_clock}))
    waits = list(d0.ins.sync_info.on_wait)
    if len(waits) > _MAXW:
        del d0.ins.sync_info.on_wait[_MAXW:]
        rest = waits[_MAXW:]
        for i in range(0, len(rest), _MAXW):
            d = self.nc.sync.drain()
            if d.ins.sync_info is None:
                d.ins.sync_info = mybir.SyncInfo(on_update=[], on_wait=[])
            d.ins.sync_info.on_wait.extend(rest[i:i + _MAXW])
    self.nc.all_engine_barrier()
    popped = self.nc._tile_sem_poison_stack.pop()
    assert popped is self._sem_poison
    self.nc.clear_and_free_semaphores(list(self.sems.allocated().values()))
    self.nc.all_engine_barrier()


def _fix_bir_json(data: bytes) -> bytes:
    m = json.loads(data)
    changed = False
    for f in m.get("functions", []):
        for b in f.get("blocks", []):
            insts = b.get("instructions")
            if not insts:
                continue
            out = []
            for inst in insts:
                si = inst.get("sync_info") or {}
                waits = si.get("on_wait") or []
                if len(waits) > 1:
                    for w in waits[:-1]:
                        _split_counter[0] += 1
                        out.append({
                            "name": f"I-sw{_split_counter[0]}",
                            "opcode": "NoOp",
                            "engine": inst.get("engine"),
                            "ins": [], "outs": [],
                            "sync_info": {"on_update": [], "on_wait": [w]},
                        })
                    si["on_wait"] = [waits[-1]]
                    changed = True
                out.append(inst)
            b["instructions"] = out
    if not changed:
        return data
    return json.dumps(m).encode()


def _install_fixes():
    TileContext._drain_and_barrier = _patched_drain_and_barrier
    if not getattr(bass.Bass, "_tilefix_json", False):
        orig = bass.Bass.to_json_bytes

        def to_json_bytes(self, *a, **k):
            return _fix_bir_json(orig(self, *a, **k))

        bass.Bass.to_json_bytes = to_json_bytes
        bass.Bass._tilefix_json = True


_install_fixes()

# ----------------------------------------------------------------------------
N_NODES = 100_000
N_EDGES = 1_600_000
F_IN = 128
H1, C1 = 2, 64
H2, C2 = 1, 64
NCORES = 8
P = 128
NBLK_GLOBAL = (N_NODES + P - 1) // P        # 782
NPAD = NBLK_GLOBAL * P                      # 100096
CH = 16                                     # stream chunk (tiles)
F32 = mybir.dt.float32
I32 = mybir.dt.int32


def _rep(v):
    """Replicate a 1-D row across 128 partitions -> [128, len] f32."""
    v = np.asarray(v, np.float32).reshape(1, -1)
    return np.ascontiguousarray(np.repeat(v, P, axis=0))


def _prep_edges(edge_index):
    src = np.asarray(edge_index[0], np.int64)
    dst = np.asarray(edge_index[1], np.int64)
    E = src.shape[0]
    order = np.argsort(dst, kind="stable")
    src_s = src[order].astype(np.int32)
    dst_s = dst[order].astype(np.int32)
    gb = dst_s // P                                    # global block id, sorted
    blk_cnt = np.bincount(gb, minlength=NBLK_GLOBAL)
    # contiguous block ranges per core, balanced by edge count
    cum = np.cumsum(blk_cnt)
    bounds = [0]
    for k in range(1, NCORES):
        t = E * k / NCORES
        b = int(np.searchsorted(cum, t))
        bounds.append(max(min(b, NBLK_GLOBAL - (NCORES - k)), bounds[-1] + 1))
    bounds.append(NBLK_GLOBAL)
    core_rng = [(bounds[k], bounds[k + 1]) for k in range(NCORES)]
    NBLKC = max(b1 - b0 for b0, b1 in core_rng)
    # tiles per block slot = max over cores
    T_list = []
    for s in range(NBLKC):
        mx = 1
        for b0, b1 in core_rng:
            if b0 + s < b1:
                mx = max(mx, (int(blk_cnt[b0 + s]) + P - 1) // P)
        T_list.append(mx + 1)  # leading all-pad tile per block (see _build_layer)
    T_total = sum(T_list)
    Tpad = ((T_total + CH - 1) // CH) * CH
    blk_start = np.concatenate([[0], cum]).astype(np.int64)  # edge offset per block
    idx_st = np.zeros((NCORES, P, Tpad), np.int32)
    ea_dummy = np.zeros((NCORES, P, Tpad), np.float32)
    dr_st = np.full((NCORES, P, Tpad), -1.0, np.float32)
    eorder = np.empty((NCORES,), object)
    slots = np.empty((NCORES,), object)
    for k in range(NCORES):
        b0, b1 = core_rng[k]
        col = 0
        ords, slts = [], []
        for s in range(NBLKC):
            b = b0 + s
            if b < b1:
                e0, e1 = int(blk_start[b]), int(blk_start[b + 1])
                n = e1 - e0
                sl = np.arange(n, dtype=np.int64)
                p_ = sl % P
                c_ = col + 1 + sl // P  # skip the leading pad tile
                idx_st[k, p_, c_] = src_s[e0:e1]
                dr_st[k, p_, c_] = (dst_s[e0:e1] % P).astype(np.float32)
                ords.append(order[e0:e1])
                slts.append((p_, c_))
            col += T_list[s]
        eorder[k] = ords
        slots[k] = slts
    return dict(core_rng=core_rng, NBLKC=NBLKC, T_list=T_list, Tpad=Tpad,
                idx_st=idx_st, dr_st=dr_st, ea_shape=ea_dummy.shape,
                eorder=eorder, slots=slots)


def _fill_ea(prep, edge_attr):
    ea = np.asarray(edge_attr, np.float32).reshape(-1)
    out = np.zeros((NCORES, P, prep["Tpad"]), np.float32)
    for k in range(NCORES):
        for (p_, c_), orig in zip(prep["slots"][k], prep["eorder"][k]):
            out[k, p_, c_] = ea[orig]
    return out


def _build_layer(COUT, H, NBLKC, T_list, Tpad, do_relu):
    """One GATv2 layer. Inputs (per core): xT [128,NPAD], xTl [128,NBLKC*128],
    streams idx/ea/dr, weights/consts. Output h_out [NBLKC*128, COUT]."""
    C = COUT // H
    nc = bass.Bass()
    xT = nc.dram_tensor("xT", [P, NPAD], F32, kind="ExternalInput")
    xTl = nc.dram_tensor("xTl", [P, NBLKC * P], F32, kind="ExternalInput")
    idx_d = nc.dram_tensor("idx", [P, Tpad], I32, kind="ExternalInput")
    ea_d = nc.dram_tensor("ea", [P, Tpad], F32, kind="ExternalInput")
    dr_d = nc.dram_tensor("dr", [P, Tpad], F32, kind="ExternalInput")
    Wl_d = nc.dram_tensor("Wl", [P, COUT], F32, kind="ExternalInput")
    Wr_d = nc.dram_tensor("Wr", [P, COUT], F32, kind="ExternalInput")
    vV_d = nc.dram_tensor("vV", [P, COUT], F32, kind="ExternalInput")
    attV_d = nc.dram_tensor("attV", [P, COUT], F32, kind="ExternalInput")
    biasV_d = nc.dram_tensor("biasV", [P, COUT], F32, kind="ExternalInput")
    bWlV_d = nc.dram_tensor("bWlV", [P, COUT], F32, kind="ExternalInput")
    bWrV_d = nc.dram_tensor("bWrV", [P, COUT], F32, kind="ExternalInput")
    iotaV_d = nc.dram_tensor("iotaV", [P, P], F32, kind="ExternalInput")
    onesV_d = nc.dram_tensor("onesV", [P, 1], F32, kind="ExternalInput")
    h_out = nc.dram_tensor("h_out", [NBLKC * P, COUT], F32, kind="ExternalOutput")
    xl_full = nc.dram_tensor("xl_full", [NPAD, COUT], F32)
    xr_loc = nc.dram_tensor("xr_loc", [NBLKC * P, COUT], F32)
    AL = mybir.AluOpType
    AF = mybir.ActivationFunctionType

    with TileContext(nc) as tc:
        with (
            tc.tile_pool(name="const", bufs=1) as cp,
            tc.tile_pool(name="sbuf", bufs=6) as pool,
            tc.tile_pool(name="st", bufs=3) as sp,
            tc.tile_pool(name="eps", bufs=2) as ep,
            tc.tile_pool(name="pd", bufs=2, space="PSUM") as ppd,
            tc.tile_pool(name="pt", bufs=2, space="PSUM") as ppt,
            tc.tile_pool(name="px", bufs=2, space="PSUM") as ppx,
            tc.tile_pool(name="po", bufs=2, space="PSUM") as ppo,
        ):
            Wl = cp.tile([P, COUT], F32); nc.sync.dma_start(out=Wl[:], in_=Wl_d[:])
            Wr = cp.tile([P, COUT], F32); nc.sync.dma_start(out=Wr[:], in_=Wr_d[:])
            vV = cp.tile([P, COUT], F32); nc.sync.dma_start(out=vV[:], in_=vV_d[:])
            attV = cp.tile([P, COUT], F32); nc.sync.dma_start(out=attV[:], in_=attV_d[:])
            biasV = cp.tile([P, COUT], F32); nc.sync.dma_start(out=biasV[:], in_=biasV_d[:])
            bWlV = cp.tile([P, COUT], F32); nc.sync.dma_start(out=bWlV[:], in_=bWlV_d[:])
            bWrV = cp.tile([P, COUT], F32); nc.sync.dma_start(out=bWrV[:], in_=bWrV_d[:])
            iotaV = cp.tile([P, P], F32); nc.sync.dma_start(out=iotaV[:], in_=iotaV_d[:])
            onesV = cp.tile([P, 1], F32); nc.sync.dma_start(out=onesV[:], in_=onesV_d[:])
            ident = cp.tile([P, P], F32); make_identity(nc, ident[:])
            Szero = cp.tile([P, P], F32)
            nc.vector.tensor_scalar(out=Szero[:], in0=iotaV[:], scalar1=0.0,
                                    scalar2=None, op0=mybir.AluOpType.mult)

            # dense: xl_full = (xT.T @ Wl) + b_l ; xr_loc likewise from xTl
            for j in range(NPAD // P):
                xt = pool.tile([P, P], F32, tag="xt")
                nc.sync.dma_start(out=xt[:], in_=xT[:, j * P:(j + 1) * P])
                pd = ppd.tile([P, COUT], F32, space="PSUM")
                nc.tensor.matmul(pd[:], lhsT=xt[:], rhs=Wl[:], start=True, stop=True)
                xls = pool.tile([P, COUT], F32, tag="xls")
                nc.vector.tensor_tensor(out=xls[:], in0=pd[:], in1=bWlV[:], op=AL.add)
                nc.sync.dma_start(out=xl_full[j * P:(j + 1) * P, :], in_=xls[:])
            for s in range(NBLKC):
                xt = pool.tile([P, P], F32, tag="xt")
                nc.sync.dma_start(out=xt[:], in_=xTl[:, s * P:(s + 1) * P])
                pd = ppd.tile([P, COUT], F32, space="PSUM")
                nc.tensor.matmul(pd[:], lhsT=xt[:], rhs=Wr[:], start=True, stop=True)
                xrs = pool.tile([P, COUT], F32, tag="xls")
                nc.vector.tensor_tensor(out=xrs[:], in0=pd[:], in1=bWrV[:], op=AL.add)
                nc.sync.dma_start(out=xr_loc[s * P:(s + 1) * P, :], in_=xrs[:])

            # edge phase
            g = 0
            for s in range(NBLKC):
                xrb = pool.tile([P, COUT], F32, tag="xrb")
                nc.sync.dma_start(out=xrb[:], in_=xr_loc[s * P:(s + 1) * P, :])
                psO = ppo.tile([P, COUT + H], F32, space="PSUM")
                for t in range(T_list[s]):
                    if g % CH == 0:
                        idxc = sp.tile([P, CH], I32, tag="idxc")
                        nc.sync.dma_start(out=idxc[:], in_=idx_d[:, g:g + CH])
                        eac = sp.tile([P, CH], F32, tag="eac")
                        nc.sync.dma_start(out=eac[:], in_=ea_d[:, g:g + CH])
                        drc = sp.tile([P, CH], F32, tag="drc")
                        nc.sync.dma_start(out=drc[:], in_=dr_d[:, g:g + CH])
                    c = g % CH
                    if t == 0:
                        # leading pad tile: absorbs the first start=True PSUM
                        # accumulation (its contribution is dropped by HW);
                        # zero matmuls only - no gather, no logit pipeline.
                        for h in range(H):
                            nc.tensor.matmul(psO[:, h * C:(h + 1) * C],
                                             lhsT=Szero[:],
                                             rhs=ident[:, :C],
                                             start=True, stop=False)
                            nc.tensor.matmul(psO[:, COUT + h:COUT + h + 1],
                                             lhsT=Szero[:], rhs=onesV[:],
                                             start=True, stop=False)
                        g += 1
                        continue
                    msgA = pool.tile([P, COUT], F32, tag="msgA")
                    nc.gpsimd.indirect_dma_start(
                        out=msgA[:], out_offset=None, in_=xl_full[:, :],
                        in_offset=bass.IndirectOffsetOnAxis(ap=idxc[:, c:c + 1], axis=0))
                    S01 = pool.tile([P, P], F32, tag="S01")
                    nc.vector.tensor_scalar(out=S01[:], in0=iotaV[:],
                                            scalar1=drc[:, c:c + 1], scalar2=None,
                                            op0=AL.is_equal)
                    pT = ppt.tile([P, P], F32, space="PSUM")
                    nc.tensor.transpose(out=pT[:], in_=S01[:], identity=ident[:])
                    selD = pool.tile([P, P], F32, tag="selD")
                    nc.scalar.copy(selD[:], pT[:])
                    pXR = ppx.tile([P, COUT], F32, space="PSUM")
                    nc.tensor.matmul(pXR[:], lhsT=selD[:], rhs=xrb[:], start=True, stop=True)
                    m = pool.tile([P, COUT], F32, tag="m")
                    nc.vector.scalar_tensor_tensor(out=m[:], in0=vV[:],
                                                   scalar=eac[:, c:c + 1], in1=pXR[:],
                                                   op0=AL.mult, op1=AL.add)
                    nc.vector.tensor_tensor(out=m[:], in0=m[:], in1=msgA[:], op=AL.add)
                    tabs = pool.tile([P, COUT], F32, tag="tabs")
                    nc.scalar.activation(tabs[:], m[:], AF.Abs)
                    q = pool.tile([P, COUT], F32, tag="q")
                    nc.vector.tensor_tensor(out=q[:], in0=m[:], in1=attV[:], op=AL.mult)
                    lin = pool.tile([P, H], F32, tag="lin")
                    nc.vector.tensor_reduce(out=lin[:], in_=q[:].rearrange("p (h c) -> p h c", h=H),
                                            axis=mybir.AxisListType.X, op=AL.add)
                    u = pool.tile([P, COUT], F32, tag="u")
                    nc.vector.tensor_tensor(out=u[:], in0=tabs[:], in1=attV[:], op=AL.mult)
                    ur = pool.tile([P, H], F32, tag="ur")
                    nc.vector.tensor_reduce(out=ur[:], in_=u[:].rearrange("p (h c) -> p h c", h=H),
                                            axis=mybir.AxisListType.X, op=AL.add)
                    logit = pool.tile([P, H], F32, tag="logit")
                    nc.vector.tensor_scalar(out=logit[:], in0=lin[:], scalar1=0.6,
                                            scalar2=None, op0=AL.mult)
                    nc.vector.scalar_tensor_tensor(out=logit[:], in0=ur[:], scalar=0.4,
                                                   in1=logit[:], op0=AL.mult, op1=AL.add)
                    ex = pool.tile([P, H], F32, tag="ex")
                    nc.scalar.activation(ex[:], logit[:], AF.Exp)
                    first, last = False, (t == T_list[s] - 1)
                    for h in range(H):
                        Sh = pool.tile([P, P], F32, tag=f"Sh{h}")
                        nc.scalar.activation(Sh[:], S01[:], AF.Copy,
                                             bias=0.0, scale=ex[:, h:h + 1])
                        nc.tensor.matmul(psO[:, h * C:(h + 1) * C], lhsT=Sh[:],
                                         rhs=msgA[:, h * C:(h + 1) * C],
                                         start=first, stop=last)
                        nc.tensor.matmul(psO[:, COUT + h:COUT + h + 1], lhsT=Sh[:],
                                         rhs=onesV[:], start=first, stop=last)
                    g += 1
                den = ep.tile([P, H], F32, tag="den")
                nc.vector.tensor_scalar_max(den[:], psO[:, COUT:COUT + H], 1e-30)
                dinv = ep.tile([P, H], F32, tag="dinv")
                nc.vector.reciprocal(dinv[:], den[:])
                hsb = ep.tile([P, COUT], F32, tag="hsb")
                for h in range(H):
                    nc.vector.tensor_scalar(out=hsb[:, h * C:(h + 1) * C],
                                            in0=psO[:, h * C:(h + 1) * C],
                                            scalar1=dinv[:, h:h + 1], scalar2=None,
                                            op0=AL.mult)
                nc.vector.tensor_tensor(out=hsb[:], in0=hsb[:], in1=biasV[:], op=AL.add)
                if do_relu:
                    nc.vector.tensor_scalar_max(hsb[:], hsb[:], 0.0)
                nc.sync.dma_start(out=h_out[s * P:(s + 1) * P, :], in_=hsb[:])
    return nc


def _run_layer(nc, per_core_ins):
    res = run_bass_kernel_spmd(nc, per_core_ins, core_ids=list(range(NCORES)))
    return [r["h_out"] for r in res.results]


def _layer_inputs(prep, ea_st, xT_full, Wl, bl, Wr, br, We, att, bias, COUT, H):
    iotaV = _rep(np.arange(P, dtype=np.float32))
    onesV = np.ones((P, 1), np.float32)
    common = dict(
        Wl=np.ascontiguousarray(Wl.astype(np.float32)),
        Wr=np.ascontiguousarray(Wr.astype(np.float32)),
        vV=_rep(We.reshape(-1)),
        attV=_rep(att.reshape(-1)),
        biasV=_rep(bias),
        bWlV=_rep(bl),
        bWrV=_rep(br),
        iotaV=iotaV,
        onesV=onesV,
        xT=xT_full,
    )
    per_core = []
    NBLKC = prep["NBLKC"]
    for k in range(NCORES):
        b0, b1 = prep["core_rng"][k]
        xTl = np.zeros((P, NBLKC * P), np.float32)
        lo, hi = b0 * P, min(b1 * P, NPAD)
        w = hi - lo
        xTl[:, :w] = xT_full[:, lo:hi]
        d = dict(common)
        d["xTl"] = xTl
        d["idx"] = prep["idx_st"][k]
        d["ea"] = ea_st[k]
        d["dr"] = prep["dr_st"][k]
        per_core.append(d)
    return per_core


def _assemble(prep, outs, COUT):
    full = np.zeros((NPAD, COUT), np.float32)
    for k in range(NCORES):
        b0, b1 = prep["core_rng"][k]
        n = (b1 - b0) * P
        full[b0 * P: b1 * P, :] = outs[k][:n, :]
    return full


def kernel(x, edge_index, edge_attr,
           W1_l, b1_l, W1_r, b1_r, W1_e, att1, bias1,
           W2_l, b2_l, W2_r, b2_r, W2_e, att2, bias2):
    x = np.asarray(x, np.float32)
    prep = _prep_edges(np.asarray(edge_index))
    ea_st = _fill_ea(prep, edge_attr)

    xpad = np.zeros((NPAD, F_IN), np.float32)
    xpad[:N_NODES] = x
    xT = np.ascontiguousarray(xpad.T)

    NBLKC, T_list, Tpad = prep["NBLKC"], prep["T_list"], prep["Tpad"]

    nc1 = _build_layer(H1 * C1, H1, NBLKC, T_list, Tpad, do_relu=True)
    ins1 = _layer_inputs(prep, ea_st, xT, W1_l, b1_l, W1_r, b1_r, W1_e, att1,
                         bias1, H1 * C1, H1)
    h_slices = _run_layer(nc1, ins1)
    h_full = _assemble(prep, h_slices, H1 * C1)
    hT = np.ascontiguousarray(h_full.T)

    nc2 = _build_layer(H2 * C2, H2, NBLKC, T_list, Tpad, do_relu=False)
    ins2 = _layer_inputs(prep, ea_st, hT, W2_l, b2_l, W2_r, b2_r, W2_e, att2,
                         bias2, H2 * C2, H2)
    o_slices = _run_layer(nc2, ins2)
    out_full = _assemble(prep, o_slices, H2 * C2)
    return out_full[:N_NODES].astype(np.float32)
